# revision 34
# baseline (speedup 1.0000x reference)
"""BERT multi-head attention forward on 8 Trainium2 NeuronCores.

Sharding: tensor-parallel over heads (16 heads -> 2 per core) for the QKV
projection and attention; a per-batch AllToAll (bf16) redistributes the
attention outputs token-wise so each core computes the output projection for
its own 512-token slice.

Engine budget (per core): ACT is the floor (~140us of exp over 16.8M score
elements); everything else is arranged to hide under it:
  - ACT runs ONLY exp (scores PSUM -> bf16 SBUF, mask bias + 1/8 scale fused).
  - DVE does all PSUM evacuation (qkv bias-adds, V-transpose copies,
    softmax normalize) -- ~70us total.
  - PE: qkv (bf16), scores (f32r, the two heads' K=64 matmuls run
    concurrently in row groups 0-63/64-127), PV (bf16, M=65 with a fused
    ones-row accumulating sum-of-exp), output projection (bf16).
  - Cross-batch pipeline: qkv(b1) + V-transposes are emitted inside
    attn(b0)'s ACT-bound loop; A2A(b0) overlaps attn(b1); outproj(b0)
    overlaps attn(b1)/A2A(b1). Exposed: qkv(b0) head + A2A(b1)+outproj(b1).

Precision: x/Wqkv/Wout are bf16 (host-converted), q/k kept f32r for the
score matmuls, attention weights bf16, accumulations f32 in PSUM.
"""

import numpy as np
import ml_dtypes
from concourse import bacc, tile, bass_utils, mybir

F32 = mybir.dt.float32
F32R = mybir.dt.float32r
BF16 = mybir.dt.bfloat16
AF = mybir.ActivationFunctionType
BF16_NP = ml_dtypes.bfloat16

B, S, E, H, D = 2, 2048, 1024, 16, 64
T = B * S                  # 4096 tokens
N_CORES = 8
HPC = H // N_CORES         # 2 heads per core
TC = 512                   # t-chunk for QKV projection
QC = 512                   # query chunk in attention
KT_S = S // 128            # 16 key tiles per batch
TPB = T // B // N_CORES    # 256 tokens per core per batch (A2A block)

_CACHE = {}


def _build(k_rep=1):
    key = k_rep
    if key in _CACHE:
        return _CACHE[key]
    nc = bacc.Bacc("TRN2", target_bir_lowering=False, debug=False, num_devices=N_CORES)

    xT = nc.dram_tensor("xT", [E, T], BF16, kind="ExternalInput").ap()
    wqkvT = nc.dram_tensor("wqkvT", [E, 3 * 128], BF16, kind="ExternalInput").ap()
    bqkv_d = nc.dram_tensor("bqkv_sb", [128, 3], F32, kind="ExternalInput").ap()
    woutT = nc.dram_tensor("woutT", [E, E], BF16, kind="ExternalInput").ap()
    bout_d = nc.dram_tensor("bout_sb", [128, 8], F32, kind="ExternalInput").ap()
    abias_d = nc.dram_tensor("abias_sb", [128, B * KT_S], F32, kind="ExternalInput").ap()
    ident_d = nc.dram_tensor("ident", [128, 64], BF16, kind="ExternalInput").ap()
    chain_d = nc.dram_tensor("chain", [1, 128], F32, kind="ExternalInput").ap()

    outT_d = nc.dram_tensor("outT", [E, 2 * TPB], F32, kind="ExternalOutput").ap()
    chout_d = nc.dram_tensor("chain_out", [1, 128], F32, kind="ExternalOutput").ap()

    with tile.TileContext(nc) as tc:
        with tc.tile_pool(name="sb", bufs=1) as sb, \
             tc.tile_pool(name="ps", bufs=1, space="PSUM") as ps, \
             tc.tile_pool(name="dram", bufs=1, space="DRAM") as dram:

            # chain passthrough (timing harness hook; negligible cost)
            ch_sb = sb.tile([1, 128], F32)
            nc.sync.dma_start(ch_sb[:], chain_d[:])
            nc.vector.tensor_copy(ch_sb[:], ch_sb[:])
            nc.sync.dma_start(chout_d[:], ch_sb[:])

            # ---- constants ----
            bqkv_sb = sb.tile([128, 3], F32)
            bout_sb = sb.tile([128, 8], F32)
            abias_sb = sb.tile([128, B * KT_S], F32)
            ident_sb = sb.tile([128, 64], BF16)
            ones_sb = sb.tile([1, 64], F32R)
            nc.sync.dma_start(bqkv_sb[:], bqkv_d[:])
            nc.sync.dma_start(bout_sb[:], bout_d[:])
            nc.sync.dma_start(abias_sb[:], abias_d[:])
            nc.sync.dma_start(ident_sb[:], ident_d[:])
            nc.vector.memset(ones_sb[:].bitcast(F32), 1.0)

            # ---- weights ----
            wq_sb = [sb.tile([128, 3 * 128], BF16, name=f"wq_{e}") for e in range(8)]
            for e in range(8):
                nc.sync.dma_start(wq_sb[e][:], wqkvT[e * 128:(e + 1) * 128, :])
            wo_sb = [sb.tile([128, E], BF16, name=f"wo_{e}") for e in range(8)]
            for e in range(8):
                nc.sync.dma_start(wo_sb[e][:], woutT[e * 128:(e + 1) * 128, :])

            for _rep in range(k_rep):
                # per-batch activations (separate tiles so cross-batch
                # pipelining never aliases)
                qk = {(j, b): sb.tile([128, S], BF16, name=f"qk{j}{b}")
                      for j in range(2) for b in range(B)}
                vv = {b: sb.tile([128, S], BF16, name=f"v{b}") for b in range(B)}
                concatT = {b: sb.tile([128, S], BF16, name=f"cc{b}") for b in range(B)}
                vcat = {}

                def emit_qkv_chunk(b, ci):
                    g = b * 4 + ci
                    cols = slice(ci * TC, (ci + 1) * TC)
                    xt = [sb.tile([128, TC], BF16, name="xt", tag=f"xt{e}", bufs=2)
                          for e in range(8)]
                    for e in range(8):
                        nc.sync.dma_start(
                            xt[e][:], xT[e * 128:(e + 1) * 128, g * TC:(g + 1) * TC])
                    for j in range(3):
                        acc = ps.tile([128, TC], F32, name="acc", tag="sc", bufs=2)
                        for e in range(8):
                            nc.tensor.matmul(acc[:], wq_sb[e][:, j * 128:(j + 1) * 128],
                                             xt[e][:], start=(e == 0), stop=(e == 7))
                        dst = qk[(j, b)] if j < 2 else vv[b]
                        with nc.allow_low_precision(reason="bf16 rounding"):
                            nc.vector.tensor_scalar_add(dst[:, cols], acc[:],
                                                        bqkv_sb[:, j:j + 1])

                def emit_transpose(b, kt):
                    for h in range(HPC):
                        if (h, b) not in vcat:
                            vt = sb.tile([128, KT_S * 65], BF16, name=f"vc{h}{b}")
                            nc.vector.memset(vt[:], 1.0)
                            vcat[(h, b)] = vt
                        tp = ps.tile([128, 64], BF16, name="tp", tag="sc", bufs=2)
                        nc.tensor.transpose(
                            tp[:],
                            vv[b][64 * h:64 * h + 64, kt * 128:(kt + 1) * 128],
                            ident_sb[64 * h:64 * h + 64, 0:64])
                        nc.vector.tensor_copy(
                            vcat[(h, b)][:, kt * 65:kt * 65 + 64], tp[:])

                def emit_attn(b, qcp, inserts=()):
                    q0s = [(2 * qcp + q) * QC for q in range(2)]
                    oaug = {(h, q): ps.tile([65, QC], F32, name=f"oaug{h}{q}",
                                            tag=f"oaug{h}{q}")
                            for h in range(HPC) for q in range(2)}
                    if qcp == 0:
                        for kt in range(KT_S):
                            emit_transpose(b, kt)
                    for kt in range(KT_S):
                        exs = {}
                        for h in range(HPC):
                            scp = ps.tile([128, 2 * QC], F32, name="scp",
                                          tag="sc", bufs=2)
                            for q in range(2):
                                nc.tensor.matmul(
                                    scp[:, q * QC:(q + 1) * QC],
                                    qk[(1, b)][64 * h:64 * h + 64,
                                               kt * 128:(kt + 1) * 128],
                                    qk[(0, b)][64 * h:64 * h + 64,
                                               q0s[q]:q0s[q] + QC],
                                    start=True, stop=True)
                            ex = sb.tile([128, 2 * QC], BF16, name="ex",
                                         tag=f"ex{h}", bufs=3)
                            with nc.allow_low_precision(reason="bf16 attn weights"):
                                nc.scalar.activation(
                                    ex[:], scp[:], AF.Exp, scale=0.125,
                                    bias=abias_sb[:, b * KT_S + kt: b * KT_S + kt + 1])
                            exs[h] = ex
                        for h in range(HPC):
                            for q in range(2):
                                nc.tensor.matmul(
                                    oaug[h, q][:],
                                    vcat[(h, b)][:, kt * 65: kt * 65 + 65],
                                    exs[h][:, q * QC:(q + 1) * QC],
                                    start=(kt == 0), stop=(kt == KT_S - 1))
                    for h in range(HPC):
                        for q in range(2):
                            sr = sb.tile([1, QC], F32R, name="sr", tag="sr", bufs=2)
                            with nc.allow_low_precision(reason="f32r recip"):
                                nc.vector.reciprocal(sr[:], oaug[h, q][64:65, :])
                            rep = ps.tile([64, QC], F32, name="rep", tag="sc", bufs=2)
                            nc.tensor.matmul(rep[:], ones_sb[:], sr[:],
                                             start=True, stop=True)
                            rr = sb.tile([64, QC], F32, name="rr", tag="rr", bufs=2)
                            nc.vector.tensor_copy(rr[:], rep[:])
                            with nc.allow_low_precision(reason="bf16 concat"):
                                nc.vector.tensor_mul(
                                    concatT[b][64 * h:64 * h + 64,
                                               q0s[q]:q0s[q] + QC],
                                    oaug[h, q][0:64, :], rr[:])
                    for fn in inserts:
                        fn()

                a2a_out = {}

                def emit_a2a(b):
                    a2a_in_b = dram.tile([N_CORES * 128, TPB], BF16, name=f"ai{b}")
                    a2a_out_b = dram.tile([N_CORES * 128, TPB], BF16, name=f"ao{b}")
                    for j in range(N_CORES):
                        nc.sync.dma_start(
                            a2a_in_b[j * 128:(j + 1) * 128, :],
                            concatT[b][:, j * TPB:(j + 1) * TPB])
                    nc.gpsimd.collective_compute(
                        "AllToAll", mybir.AluOpType.bypass,
                        replica_groups=[list(range(N_CORES))],
                        ins=[a2a_in_b.opt()], outs=[a2a_out_b.opt()])
                    a2a_out[b] = a2a_out_b

                def emit_outproj(b, eos):
                    cs = [sb.tile([128, TPB], BF16, name="cs", tag=f"cs{kt}", bufs=2)
                          for kt in range(8)]
                    for kt in range(8):
                        nc.sync.dma_start(
                            cs[kt][:], a2a_out[b][kt * 128:(kt + 1) * 128, :])
                    for eo in eos:
                        facc = ps.tile([128, TPB], F32, name="facc", tag="sc", bufs=2)
                        for kt in range(8):
                            nc.tensor.matmul(facc[:],
                                             wo_sb[kt][:, eo * 128:(eo + 1) * 128],
                                             cs[kt][:], start=(kt == 0), stop=(kt == 7))
                        osb = sb.tile([128, TPB], F32, name="osb", tag="osb", bufs=2)
                        nc.vector.tensor_scalar_add(osb[:], facc[:],
                                                    bout_sb[:, eo:eo + 1])
                        nc.sync.dma_start(
                            outT_d[eo * 128:(eo + 1) * 128, b * TPB:(b + 1) * TPB],
                            osb[:])

                # ---- pipeline schedule ----
                # qkv(b1) hides inside attn(b0)'s ACT-bound windows; the b0
                # exchange overlaps attn(b1); outproj(b0) overlaps the b1
                # exchange. Exposed: qkv(b0) head + a2a(b1)+outproj(b1) tail.
                for ci in range(4):
                    emit_qkv_chunk(0, ci)
                emit_attn(0, 0, [lambda: emit_qkv_chunk(1, 0)])
                emit_attn(0, 1, [lambda: emit_qkv_chunk(1, 1),
                                 lambda: emit_qkv_chunk(1, 2),
                                 lambda: emit_qkv_chunk(1, 3)])
                emit_a2a(0)
                emit_attn(1, 0)
                emit_attn(1, 1, [lambda: emit_outproj(0, range(8))])
                emit_a2a(1)
                emit_outproj(1, range(8))

    nc.compile()
    _CACHE[key] = nc
    return nc


def _host_prep(x, mask, Wqkv, bqkv, Wout, bout):
    x = np.ascontiguousarray(np.asarray(x, np.float32))
    Wqkv = np.asarray(Wqkv, np.float32)
    bqkv = np.asarray(bqkv, np.float32)
    Wout = np.asarray(Wout, np.float32)
    bout = np.asarray(bout, np.float32)
    mask = np.asarray(mask)

    xT = np.ascontiguousarray(x.reshape(T, E).T.astype(BF16_NP))          # [E, T]
    m = mask.reshape(B, S)
    ab = np.where(m == 0, np.float32(-30000.0), np.float32(0.0)).astype(np.float32)
    abias_sb = np.ascontiguousarray(ab.reshape(B, KT_S, 128).transpose(2, 0, 1)
                                    .reshape(128, B * KT_S))
    woutT = np.ascontiguousarray(Wout.T.astype(BF16_NP))                  # [e_in, e_out]
    bout_sb = np.ascontiguousarray(bout.reshape(8, 128).T)
    ident = np.vstack([np.eye(64, dtype=np.float32)] * 2).astype(BF16_NP)
    chain = np.zeros((1, 128), np.float32)

    in_maps = []
    for c in range(N_CORES):
        hs = [HPC * c + i for i in range(HPC)]
        rows = []
        for tix in range(3):  # q, k, v
            for h in hs:
                rows.append(Wqkv[tix * E + h * D: tix * E + (h + 1) * D])
        Wc = np.concatenate(rows, axis=0)                              # [384, 1024]
        wqkvT_c = np.ascontiguousarray(Wc.T.astype(BF16_NP))           # [1024, 384]
        brows = []
        for tix in range(3):
            for h in hs:
                brows.append(bqkv[tix * E + h * D: tix * E + (h + 1) * D])
        bq_c = np.concatenate(brows).reshape(3, 128).T                 # [128, 3]
        in_maps.append({
            "xT": xT, "wqkvT": wqkvT_c, "bqkv_sb": np.ascontiguousarray(bq_c),
            "woutT": woutT, "bout_sb": bout_sb, "abias_sb": abias_sb,
            "ident": ident, "chain": chain,
        })
    return in_maps


def _assemble(results):
    out = np.empty((B, S, E), np.float32)
    for c in range(N_CORES):
        outT_c = results[c]["outT"]                                    # [E, 2*TPB]
        for b in range(B):
            out[b, c * TPB:(c + 1) * TPB, :] = outT_c[:, b * TPB:(b + 1) * TPB].T
    return out


def kernel(x, mask, Wqkv, bqkv, Wout, bout):
    nc = _build()
    in_maps = _host_prep(x, mask, Wqkv, bqkv, Wout, bout)
    res = bass_utils.run_bass_kernel_spmd(nc, in_maps, core_ids=list(range(N_CORES)))
    return _assemble(res.results)
